# revision 20
# baseline (speedup 1.0000x reference)
"""Trainium2 Bass kernel for nn_Polynomial: out = poly_basis(x) @ W.T + bias.

fp16 product-basis pipeline, pure data parallel over 8 cores (62592 rows
each, padded; 9 supertiles of 128x56 rows):
  - basis built in SBUF fp16 in two tiles: bA [128,G,128] holds x, pairs and
    triples k=0..6; bB [128,G,64] holds the k=7 triples + the constant bias
    column (+ zero pad). Broadcast multiplies split between DVE and Pool.
  - per 16-group superblock: 16 A-transposes ([128,128] fp16, 1 cyc/row) and
    4 packed B-transposes (two 64-col group slabs land at partitions 0/64),
    DVE evacuates A at fp16 2x, evacB alternates ACT/DVE, 16 accumulated
    fp16 matmuls against the permuted weight tables (wbr replicated at
    partition 64 so the inferred PE tile_position stays legal), ACT
    evacuates the output as fp16 (host casts back to f32; halves the
    output DMA traffic).
  - output DMA hedged across SP/ACT HWDGE queues + Pool SWDGE.
"""

import numpy as np

import concourse.bass as bass
import concourse.bacc as bacc
import concourse.mybir as mybir
from concourse import bass_utils
from concourse import tile

IN_F = 8
OUT_F = 64
K_TOT = 165   # 164 monomials + 1 const column (fused bias)
KA = 128      # chunk-a columns (transposed as one 128x128 block)
KB = K_TOT - KA  # 37
BW = 192      # basis tile free width: 128 A-cols + 64-col B slab (37 live)

N_CORES = 8
N_ROWS = 500000
RPC_RAW = N_ROWS // N_CORES          # 62500
G_MAIN = 56
# uniform 9x56 supertiles (with the fp16 output the drain is cheap enough
# that tapering no longer pays); 9*56 = 504 groups = 64512 rows
G_LIST = [G_MAIN] * 9
ROWS_PER_CORE = 128 * sum(G_LIST)    # 62592
SB = 8                               # superblock groups

F32 = mybir.dt.float32
F16 = mybir.dt.float16


def _pair_off(j: int) -> int:
    return j * (j + 1) // 2


def _trip_off(k: int) -> int:
    return k * (k + 1) * (k + 2) // 6


# Basis column layout (165 live columns):
#   [0..8)    x_i
#   [8..44)   x_i * x_j      (i<=j), col = 8 + _pair_off(j) + i
#   [44..164) x_i x_j x_k    (i<=j<=k), col = 44 + _trip_off(k) + _pair_off(j) + i
#   [164]     1.0 (bias column)
#   [165..192) pad (zeroed once)


def _term_col(e) -> int:
    facs = []
    for f in range(IN_F):
        facs += [f] * int(e[f])
    if len(facs) == 1:
        return facs[0]
    if len(facs) == 2:
        i, j = facs
        return 8 + _pair_off(j) + i
    i, j, k = facs
    return 44 + _trip_off(k) + _pair_off(j) + i


def _exponents() -> np.ndarray:
    deg = np.arange(4)
    comb = np.array(np.meshgrid(*([deg] * IN_F))).T.reshape(-1, IN_F)
    s = comb.sum(axis=1)
    nz = (comb != 0).sum(axis=1)
    keep = ((nz == 1) & (s <= 3)) | ((nz > 1) & (s <= 3))
    return comb[keep].astype(np.int32)


def make_weight_tables(weight: np.ndarray, bias: np.ndarray):
    """Permute reference weight [64, 164] into the on-chip column order.
    Returns (wa [128, 64] fp16, wbr [128, 64] fp16) where wbr holds the
    37-row B block replicated at partition offsets 0 and 64."""
    E = _exponents()
    wt = np.zeros((K_TOT, OUT_F), np.float32)
    for t in range(E.shape[0]):
        wt[_term_col(E[t])] += weight[:, t].astype(np.float32)
    wt[K_TOT - 1] = bias.astype(np.float32)
    wa = wt[0:KA].astype(np.float16)
    wbr = np.zeros((128, OUT_F), np.float16)
    wbr[0:KB] = wt[KA:K_TOT].astype(np.float16)
    wbr[64:64 + KB] = wt[KA:K_TOT].astype(np.float16)
    return wa, wbr


def poly2_tile_kernel(tc, x_ap, wa_ap, wbr_ap, id_ap, out_ap,
                      g_list=G_LIST, bench_reps=None):
    nc = tc.nc
    from contextlib import ExitStack

    gmax = max(g_list)

    with ExitStack() as ctx:
        cpool = ctx.enter_context(tc.tile_pool(name="cpool", bufs=1))
        xpool = ctx.enter_context(tc.tile_pool(name="xpool", bufs=3))
        bpool = ctx.enter_context(tc.tile_pool(name="bpool", bufs=3))
        spoolA = ctx.enter_context(tc.tile_pool(name="spoolA", bufs=4))
        spoolB = ctx.enter_context(tc.tile_pool(name="spoolB", bufs=4))
        opool = ctx.enter_context(tc.tile_pool(name="opool", bufs=3))
        pstA = ctx.enter_context(tc.tile_pool(name="pstA", bufs=2, space="PSUM"))
        pstB = ctx.enter_context(tc.tile_pool(name="pstB", bufs=2, space="PSUM"))
        pstO = ctx.enter_context(tc.tile_pool(name="pstO", bufs=2, space="PSUM"))

        ident = cpool.tile([128, 128], F16)
        wa = cpool.tile([128, OUT_F], F16)
        wbr = cpool.tile([128, OUT_F], F16)
        nc.scalar.dma_start(out=ident[:], in_=id_ap)
        nc.scalar.dma_start(out=wa[:], in_=wa_ap)
        nc.scalar.dma_start(out=wbr[:], in_=wbr_ap)

        # Prologue: pre-touch all basis buffers; bias col = 1, pad cols = 0.
        # Steady-state iterations only write live columns, so these persist.
        for _ in range(3):
            bB = bpool.tile([128, gmax, 64], F16, tag="bB")
            nc.gpsimd.memset(bB[:, :, 36:64], 0.0)
            nc.gpsimd.memset(bB[:, :, 36:37], 1.0)

        def do_supertile(row_off, g, ti=0):
            xv = x_ap[row_off:row_off + 128 * g].rearrange("(p g) f -> p g f", g=g)
            ov = out_ap[row_off:row_off + 128 * g].rearrange("(p g) f -> p g f", g=g)

            x3 = xpool.tile([128, g, IN_F], F16, tag="x3")
            nc.sync.dma_start(out=x3[:], in_=xv)

            bA = bpool.tile([128, gmax, 128], F16, tag="bA")
            bB = bpool.tile([128, gmax, 64], F16, tag="bB")
            # build in group-chunks so the first superblocks can start while
            # later chunks are still being built; the first supertile uses
            # quarter-chunks to fill the pipeline faster
            nchunk = 4 if ti == 0 else 2
            bounds = [(i * g // nchunk, (i + 1) * g // nchunk)
                      for i in range(nchunk)]
            for g0, g1 in bounds:
                gh = g1 - g0
                # x columns (f32 -> fp16) on DVE (2x_2p all-SBUF copy)
                nc.vector.tensor_copy(out=bA[:, g0:g1, 0:IN_F], in_=x3[:, g0:g1])
                # pairs on Pool (otherwise idle); in the very first chunk
                # split them across DVE+Pool to shorten the cold-start chain
                for j in range(IN_F):
                    w_ = j + 1
                    o = 8 + _pair_off(j)
                    peng = (nc.vector if (ti == 0 and g0 == 0 and j % 2 == 0)
                            else nc.gpsimd)
                    peng.tensor_mul(
                        out=bA[:, g0:g1, o:o + w_],
                        in0=bA[:, g0:g1, 0:w_],
                        in1=bA[:, g0:g1, j:j + 1].broadcast_to([128, gh, w_]),
                    )
                # triples k=0..6 fill bA cols 44..128; k=7 fills bB cols 0..36.
                # Small blocks (k<4) on DVE, the four big ones on Pool.
                for k in range(IN_F):
                    w_ = _pair_off(k + 1)
                    eng = nc.vector if k < 4 else nc.gpsimd
                    dst = (bA[:, g0:g1, 44 + _trip_off(k):44 + _trip_off(k) + w_]
                           if k < 7 else bB[:, g0:g1, 0:w_])
                    eng.tensor_mul(
                        out=dst,
                        in0=bA[:, g0:g1, 8:8 + w_],
                        in1=bA[:, g0:g1, k:k + 1].broadcast_to([128, gh, w_]),
                    )

            out3 = opool.tile([128, g, OUT_F], F16, tag="out3")

            for q0 in range(0, g, 2 * SB):
                sbg = min(2 * SB, g - q0)      # 16-group superblock (tail 8)
                npair = sbg // 2
                psA = pstA.tile([128, 2 * SB, 128], F16, tag="psA")
                psB = pstB.tile([128, SB, 128], F16, tag="psB")
                for qi in range(sbg):
                    nc.tensor.transpose(psA[:, qi, :], bA[:, q0 + qi, :], ident[:])
                for pj in range(npair):
                    nc.tensor.transpose(
                        psB[:, pj, :], bB[:, q0 + 2 * pj:q0 + 2 * pj + 2, :],
                        ident[:])
                sbA = spoolA.tile([128, 2 * SB, 128], F16, tag="sbA")
                sbB = spoolB.tile([128, SB, 128], F16, tag="sbB")
                nc.vector.tensor_copy(out=sbA[:, 0:sbg, :], in_=psA[:, 0:sbg, :])
                beng = nc.scalar if (q0 // (2 * SB)) % 4 < 1 + (ti % 2) else nc.vector
                if beng is nc.scalar:
                    beng.copy(out=sbB[:, 0:npair, :], in_=psB[:, 0:npair, :])
                else:
                    beng.tensor_copy(out=sbB[:, 0:npair, :], in_=psB[:, 0:npair, :])
                for h0 in range(0, sbg, SB):
                    hn = min(SB, sbg - h0)
                    pso = pstO.tile([128, SB, OUT_F], F32, tag="pso")
                    for qi2 in range(hn):
                        qi = h0 + qi2
                        pj, sl = divmod(qi, 2)
                        nc.tensor.matmul(pso[:, qi2, :], lhsT=sbA[:, qi, :],
                                         rhs=wa[:], start=True, stop=False)
                        nc.tensor.matmul(pso[:, qi2, :],
                                         lhsT=sbB[64 * sl:64 * sl + KB, pj, :],
                                         rhs=wbr[64 * sl:64 * sl + KB, :],
                                         start=False, stop=True)
                    nc.scalar.copy(out=out3[:, q0 + h0:q0 + h0 + hn, :],
                                   in_=pso[:, 0:hn, :])

            # output DMA hedged across the SP/ACT HWDGE queues + Pool SWDGE
            # (real-HW per-queue bandwidth may bind; sim favors SP-only).
            # Two rounds per supertile so the drain starts at mid-supertile.
            for h0, h1 in ((0, g // 2), (g // 2, g)):
                hw = h1 - h0
                c1 = h0 + (3 * hw) // 8
                c2 = h0 + (3 * hw) // 4
                nc.sync.dma_start(out=ov[:, h0:c1], in_=out3[:, h0:c1])
                nc.scalar.dma_start(out=ov[:, c1:c2], in_=out3[:, c1:c2])
                nc.gpsimd.dma_start(out=ov[:, c2:h1], in_=out3[:, c2:h1])

        if bench_reps is None:
            off = 0
            for ti, g in enumerate(g_list):
                do_supertile(off, g, ti)
                off += 128 * g
        else:
            with tc.For_i(0, bench_reps, 1):
                do_supertile(0, g_list[0])


_CACHED_NC = {}


def build_nc(g_list=None, bench_reps=None):
    if g_list is None:
        g_list = G_LIST
    key = (tuple(g_list), bench_reps)
    if key not in _CACHED_NC:
        rows = 128 * sum(g_list)
        nc = bacc.Bacc("TRN2", target_bir_lowering=False, debug=False,
                       num_devices=N_CORES)
        x_d = nc.dram_tensor("x", [rows, IN_F], F16, kind="ExternalInput")
        wa_d = nc.dram_tensor("wa", [128, OUT_F], F16, kind="ExternalInput")
        wbr_d = nc.dram_tensor("wbr", [128, OUT_F], F16, kind="ExternalInput")
        id_d = nc.dram_tensor("ident", [128, 128], F16, kind="ExternalInput")
        o_d = nc.dram_tensor("out", [rows, OUT_F], F16, kind="ExternalOutput")
        with tile.TileContext(nc) as tc:
            poly2_tile_kernel(tc, x_d.ap(), wa_d.ap(), wbr_d.ap(), id_d.ap(),
                              o_d.ap(), g_list=g_list, bench_reps=bench_reps)
        nc.compile()
        _CACHED_NC[key] = nc
    return _CACHED_NC[key]


# sim.py compatibility
ROWS_PER_SUPER = 128 * G_MAIN
N_SUPER = len(G_LIST)

_IDENT = np.eye(128, dtype=np.float16)


def kernel(x, weight, bias):
    x = np.ascontiguousarray(np.asarray(x, dtype=np.float32))
    wa, wbr = make_weight_tables(np.asarray(weight, np.float32),
                                 np.asarray(bias, np.float32))
    nc = build_nc()

    in_maps = []
    for c in range(N_CORES):
        shard = x[c * RPC_RAW:(c + 1) * RPC_RAW]
        xpad = np.zeros((ROWS_PER_CORE, IN_F), np.float16)
        xpad[:shard.shape[0]] = shard.astype(np.float16)
        in_maps.append({"x": xpad, "wa": wa, "wbr": wbr, "ident": _IDENT})

    res = bass_utils.run_bass_kernel_spmd(nc, in_maps, core_ids=list(range(N_CORES)))
    outs = [r["out"][:RPC_RAW].astype(np.float32) for r in res.results]
    return np.concatenate(outs, axis=0)
